# revision 4
# baseline (speedup 1.0000x reference)
"""MoE (dense-router, top-2 of 8 experts) Trainium2 kernel.

Strategy: expert-parallel over 8 NeuronCores.
  - Router (softmax + top-2 over E=8) is tiny (~140 MFLOP) and runs on host;
    it bit-matches jax.lax.top_k on the reference input distribution.
  - Tokens are dispatched (gathered) per expert on host; each core runs ONE
    expert's FFN on its gathered tokens:  Y^T = W2^T @ gelu(W1^T @ X^T + b1) + b2
    computed fully transposed so W1 [D,H] and W2 [H,D] feed the PE as lhsT
    unchanged, and biases are per-partition scalars.
  - Per-token top-2 combine weights are applied during host scatter-add.

Shapes are hardcoded for the fixed problem: B=4,S=2048,D=1024,E=8,K=2,H=4096.
"""

import numpy as np
import ml_dtypes

B, S, D, E, TOPK = 4, 2048, 1024, 8, 2
H = 4 * D
P = 128
NC = 512  # matmul moving free dim (one PSUM bank of fp32)
KD = D // P   # 8
KH = H // P   # 32

_CACHE = {}


def _build(C):
    import concourse.mybir as mybir
    import concourse.tile as tile
    from concourse import bacc
    from concourse.bass import ds, ts
    from contextlib import ExitStack

    NT = C // NC
    f32, bf16 = mybir.dt.float32, mybir.dt.bfloat16
    Gelu = mybir.ActivationFunctionType.Gelu

    nc = bacc.Bacc("TRN2", target_bir_lowering=False, debug=False)
    xt = nc.dram_tensor("xt", [D, C], bf16, kind="ExternalInput").ap()
    w1 = nc.dram_tensor("w1", [D, H], bf16, kind="ExternalInput").ap()
    w2 = nc.dram_tensor("w2", [H, D], bf16, kind="ExternalInput").ap()
    b1t = nc.dram_tensor("b1t", [P, KH], f32, kind="ExternalInput").ap()
    b2t = nc.dram_tensor("b2t", [P, KD], f32, kind="ExternalInput").ap()
    yt = nc.dram_tensor("yt", [D, C], f32, kind="ExternalOutput").ap()

    with tile.TileContext(nc) as tc, ExitStack() as ctx:
        consts = ctx.enter_context(tc.tile_pool(name="consts", bufs=1))
        xt_pool = ctx.enter_context(tc.tile_pool(name="xtp", bufs=2))
        ht_pool = ctx.enter_context(tc.tile_pool(name="htp", bufs=36))
        out_pool = ctx.enter_context(tc.tile_pool(name="outp", bufs=4))
        ps1 = ctx.enter_context(tc.tile_pool(name="ps1", bufs=4, space="PSUM"))
        ps2 = ctx.enter_context(tc.tile_pool(name="ps2", bufs=4, space="PSUM"))

        b1_sb = consts.tile([P, KH], f32, tag="b1")
        nc.sync.dma_start(b1_sb[:], b1t[:, :])
        b2_sb = consts.tile([P, KD], f32, tag="b2")
        nc.sync.dma_start(b2_sb[:], b2t[:, :])

        # W1 resident: 8 tiles [128, H] (partition = d-block)
        w1_sb = []
        for kd in range(KD):
            t = consts.tile([P, H], bf16, tag=f"w1_{kd}")
            nc.sync.dma_start(t[:], w1[ds(kd * P, P), :])
            w1_sb.append(t)

        w2_sb = [None] * KH  # loaded after chunk-0 mm1 is emitted (DMA ordering)

        def load_w2():
            for kh in range(KH):
                t = consts.tile([P, D], bf16, tag=f"w2_{kh}")
                nc.sync.dma_start(t[:], w2[ds(kh * P, P), :])
                w2_sb[kh] = t

        for c in range(NT):
            xts = []
            for kd in range(KD):
                t = xt_pool.tile([P, NC], bf16, tag=f"xt{kd}")
                nc.sync.dma_start(t[:], xt[ds(kd * P, P), ts(c, NC)])
                xts.append(t)

            # mm1: H^T[h,c] = gelu(sum_d W1[d,h] * X^T[d,c] + b1[h])
            hts = []
            for hi in range(KH):
                ps = ps1.tile([P, NC], f32, tag="ps1")
                for kd in range(KD):
                    nc.tensor.matmul(
                        ps[:],
                        w1_sb[kd][:, ds(hi * P, P)],
                        xts[kd][:],
                        start=(kd == 0),
                        stop=(kd == KD - 1),
                    )
                h = ht_pool.tile([P, NC], bf16, tag="ht")
                nc.scalar.activation(h[:], ps[:], Gelu, bias=b1_sb[:, ds(hi, 1)])
                hts.append(h)

            if c == 0:
                load_w2()

            # mm2: Y^T[d,c] = sum_h W2[h,d] * H^T[h,c] + b2[d]
            for dt in range(KD):
                ps = ps2.tile([P, NC], f32, tag="ps2")
                for kh in range(KH):
                    nc.tensor.matmul(
                        ps[:],
                        w2_sb[kh][:, ds(dt * P, P)],
                        hts[kh][:],
                        start=(kh == 0),
                        stop=(kh == KH - 1),
                    )
                o = out_pool.tile([P, NC], f32, tag="out")
                nc.vector.tensor_scalar_add(o[:], ps[:], b2_sb[:, ds(dt, 1)])
                nc.sync.dma_start(yt[ds(dt * P, P), ts(c, NC)], o[:])

    nc.compile()
    return nc


def _route(x, Wg, bg):
    """Host router: softmax logits + top-2 (torch.topk tie semantics)."""
    xf = x.reshape(B * S, D)
    logits = xf @ Wg + bg
    logits -= logits.max(-1, keepdims=True)
    p = np.exp(logits)
    p /= p.sum(-1, keepdims=True)
    idx = np.argsort(-p, axis=-1, kind="stable")[:, :TOPK]
    w = np.take_along_axis(p, idx, axis=-1)
    return xf, idx, w


def _prepare(inputs):
    """Host router + dispatch. Returns (nc, in_maps, meta) where meta is
    what _combine needs to scatter per-expert outputs back."""
    x = np.asarray(inputs["x"], dtype=np.float32)
    Wg = np.asarray(inputs["Wg"], dtype=np.float32)
    bg = np.asarray(inputs["bg"], dtype=np.float32)
    W1 = np.asarray(inputs["W1"], dtype=np.float32)
    b1 = np.asarray(inputs["b1"], dtype=np.float32)
    W2 = np.asarray(inputs["W2"], dtype=np.float32)
    b2 = np.asarray(inputs["b2"], dtype=np.float32)

    xf, idx, w = _route(x, Wg, bg)

    toks, wts = [], []
    for e in range(E):
        te, ke = np.nonzero(idx == e)
        toks.append(te)
        wts.append(w[te, ke])
    maxn = max(len(t) for t in toks)
    C = max(NC, -(-maxn // NC) * NC)

    bf16 = ml_dtypes.bfloat16
    in_maps = []
    for e in range(E):
        ne = len(toks[e])
        XT = np.zeros((D, C), dtype=bf16)
        XT[:, :ne] = xf[toks[e]].T.astype(bf16)
        in_maps.append(
            {
                "xt": XT,
                "w1": W1[e].astype(bf16),
                "w2": W2[e].astype(bf16),
                "b1t": np.ascontiguousarray(b1[e].reshape(KH, P).T),
                "b2t": np.ascontiguousarray(b2[e].reshape(KD, P).T),
            }
        )

    nc = _CACHE.get(C)
    if nc is None:
        nc = _CACHE[C] = _build(C)

    return nc, in_maps, (toks, wts)


def _combine(results, meta):
    toks, wts = meta
    y = np.zeros((B * S, D), dtype=np.float32)
    for e in range(E):
        ne = len(toks[e])
        YT = results[e]["yt"]  # [D, C] f32
        y[toks[e]] += wts[e][:, None] * YT[:, :ne].T
    return y.reshape(B, S, D)


def kernel(**inputs):
    from concourse.bass_utils import run_bass_kernel_spmd

    nc, in_maps, meta = _prepare(inputs)
    res = run_bass_kernel_spmd(nc, in_maps, core_ids=list(range(E)))
    return _combine(res.results, meta)
